# revision 9
# baseline (speedup 1.0000x reference)
"""Trainium2 Bass kernel for a top-2-of-8 MoE layer (B=4, T=2048, D=1024, F=4096).

Strategy: expert-parallel across the 8 NeuronCores. The router (tiny
8192x1024 @ 1024x8 gemm + softmax + top-2) runs on host; it determines the
token->expert dispatch, i.e. the sharding. Each core is assigned one expert
and receives that expert's routed tokens gathered and pre-transposed to
[D, C] layout, plus the expert's w1/b1/w2. On-device, each core runs the
expert FFN for its tokens: h = gelu(x @ w1 + b1); y = h @ w2. The F=4096
dimension is split into 4 resident-weight passes so each pass's weight slabs
(w1[:, fq], w2[fq, :] = 8.4 MB) stay in SBUF while all tokens stream
through; each pass emits a partial y. Host sums the 4 partials, adds b2,
scales by the top-2 combine weight and scatter-adds into the full output.

Matmuls run as float32r (fp32 operands truncated to ~fp22 inside the PE at
full bf16 rate); accumulation is fp32 in PSUM.
"""

import math

import numpy as np

import concourse.bacc as bacc
import concourse.mybir as mybir
import concourse.tile as tile
from concourse import bass_utils

B, T, D, F, E, TOP_K = 4, 2048, 1024, 4096, 8, 2
NT = B * T                      # 8192 tokens
P = 128                         # partitions
NDT = D // P                    # 8 d-tiles
NPASS = 4                       # F split into 4 weight-resident passes
FQ = F // NPASS                 # 1024 features per pass
NFT = FQ // P                   # 8 f-tiles per pass
TB = 512                        # token block (<= one fp32 PSUM bank)

MM_DT = mybir.dt.float32r      # matmul operand dtype (float32r or float32)
F32 = mybir.dt.float32

# Test hooks: when TRACE is set, the SPMD run captures an NTFF profile and the
# BassKernelResults lands in LAST for exec-time inspection.
TRACE = False
TRACE_CORES = [0]
LAST = None


def _blocks(c):
    # Ragged (sub-TB) block first: the pass then ENDS with a full block whose
    # GEMM2 (~27us) covers the next pass's w1 slab reload (~12us).
    out, t0 = [], 0
    r = c % TB
    if r:
        out.append((0, r))
        t0 = r
    while t0 < c:
        out.append((t0, TB))
        t0 += TB
    return out


def build_program(c):
    """One SPMD program; each core runs it on its own expert's data."""
    nc = bacc.Bacc("TRN2", target_bir_lowering=False, debug=False, num_devices=E)

    xt = nc.dram_tensor("xt", [D, c], MM_DT, kind="ExternalInput").ap()
    w1 = nc.dram_tensor("w1", [D, F], MM_DT, kind="ExternalInput").ap()
    b1 = nc.dram_tensor("b1", [P, F // P], F32, kind="ExternalInput").ap()
    w2 = nc.dram_tensor("w2", [F, D], MM_DT, kind="ExternalInput").ap()
    y = nc.dram_tensor("y", [NPASS, D, c], F32, kind="ExternalOutput").ap()

    xt_r = xt.rearrange("(a p) c -> p a c", p=P)    # [128, 8, c]

    with tile.TileContext(nc) as tc:
        with (
            tc.tile_pool(name="w1pool", bufs=2) as w1pool,
            tc.tile_pool(name="w2pool", bufs=1) as w2pool,
            tc.tile_pool(name="bpool", bufs=2) as bpool,
            tc.tile_pool(name="xpool", bufs=2) as xpool,
            tc.tile_pool(name="hpool", bufs=2) as hpool,
            tc.tile_pool(name="ypool", bufs=3) as ypool,
            tc.tile_pool(name="psh", bufs=2, space="PSUM") as pshp,
            tc.tile_pool(name="psy", bufs=4, space="PSUM") as psyp,
        ):
            for p in range(NPASS):
                # Resident weight slabs for this pass's feature quarter.
                # w1 on the sync HWDGE ring; w2 follows it (needed ~27us
                # later); xts rides the scalar HWDGE ring in parallel.
                w1s = w1pool.tile([P, NDT, FQ], MM_DT)      # [128, di, f]
                for di in range(NDT):
                    nc.sync.dma_start(
                        w1s[:, di, :],
                        w1[di * P:(di + 1) * P, p * FQ:(p + 1) * FQ],
                    )
                b1s = bpool.tile([P, NFT], F32)
                nc.gpsimd.dma_start(b1s[:], b1[:, p * NFT:(p + 1) * NFT])
                # w2 rides the gpsimd SWDGE ring: its slot-reuse wait (all of
                # the previous pass's G2 matmuls) must not block w1/xts issue
                # on the other rings. It is only needed ~27us into the pass.
                w2s = w2pool.tile([P, NFT, D], MM_DT)       # [128, fi, d]
                for fi in range(NFT):
                    nc.gpsimd.dma_start(
                        w2s[:, fi, :],
                        w2[p * FQ + fi * P:p * FQ + (fi + 1) * P, :],
                    )

                for (t0, tb) in _blocks(c):
                    xts = xpool.tile([P, NDT, tb], MM_DT, tag="xts")
                    nc.scalar.dma_start(xts[:], xt_r[:, :, t0:t0 + tb])

                    h_all = hpool.tile([P, NFT, tb], MM_DT, tag="h")
                    for fi in range(NFT):
                        psh = pshp.tile([P, tb], F32, tag="psh")
                        for di in range(NDT):
                            nc.tensor.matmul(
                                psh[:],
                                w1s[:, di, fi * P:(fi + 1) * P],
                                xts[:, di, :],
                                start=(di == 0),
                                stop=(di == NDT - 1),
                            )
                        nc.scalar.activation(
                            h_all[:, fi, :],
                            psh[:],
                            mybir.ActivationFunctionType.Gelu,
                            bias=b1s[:, fi:fi + 1],
                        )

                    for dj in range(NDT):
                        psy = psyp.tile([P, tb], F32, tag="psy")
                        for fi in range(NFT):
                            nc.tensor.matmul(
                                psy[:],
                                w2s[:, fi, dj * P:(dj + 1) * P],
                                h_all[:, fi, :],
                                start=(fi == 0),
                                stop=(fi == NFT - 1),
                            )
                        yt = ypool.tile([P, tb], F32, tag="yt")
                        nc.vector.tensor_copy(yt[:], psy[:])
                        nc.gpsimd.dma_start(
                            y[p, dj * P:(dj + 1) * P, t0:t0 + tb], yt[:]
                        )

    nc.compile()
    return nc


def _route(xf, gate_w):
    """Host router: softmax top-2, reference-equivalent in fp32."""
    logits = xf @ gate_w                                   # [NT, E] f32
    m = logits.max(axis=-1, keepdims=True)
    p = np.exp(logits - m, dtype=np.float32)
    p /= p.sum(axis=-1, keepdims=True)
    idx = np.argsort(-p, axis=-1, kind="stable")[:, :TOP_K]  # matches lax.top_k
    wtop = np.take_along_axis(p, idx, axis=-1)
    wtop = wtop / (wtop.sum(axis=-1, keepdims=True) + 1e-9)
    return p, idx, wtop


def kernel(x, gate_w, w1, b1, w2, b2):
    x = np.ascontiguousarray(x, dtype=np.float32)
    gate_w = np.ascontiguousarray(gate_w, dtype=np.float32)
    w1 = np.ascontiguousarray(w1, dtype=np.float32)
    b1 = np.ascontiguousarray(b1, dtype=np.float32)
    w2 = np.ascontiguousarray(w2, dtype=np.float32)
    b2 = np.ascontiguousarray(b2, dtype=np.float32)

    xf = x.reshape(NT, D)
    probs, idx, wtop = _route(xf, gate_w)

    toks, cws = [], []
    for e in range(E):
        sel = (idx[:, 0] == e) | (idx[:, 1] == e)
        tok = np.nonzero(sel)[0]
        cw = np.where(idx[:, 0] == e, wtop[:, 0], wtop[:, 1])[tok]
        toks.append(tok)
        cws.append(cw.astype(np.float32))

    max_n = max(len(t) for t in toks)
    c = max(512, int(math.ceil(max_n / 256.0)) * 256)

    xfT = np.ascontiguousarray(xf.T)                    # [D, NT]
    in_maps = []
    for e in range(E):
        n = len(toks[e])
        xt_e = np.zeros((D, c), dtype=np.float32)
        xt_e[:, :n] = xfT[:, toks[e]]
        in_maps.append({
            "xt": xt_e,
            "w1": w1[e],
            "b1": np.ascontiguousarray(b1[e].reshape(F // P, P).T),
            "w2": w2[e],
        })

    nc = build_program(c)
    global LAST
    res = bass_utils.run_bass_kernel_spmd(
        nc,
        in_maps,
        core_ids=list(range(E)),
        trace=TRACE,
        trace_cores=TRACE_CORES if TRACE else None,
    )
    LAST = res

    outT = np.zeros((D, NT), dtype=np.float32)
    for e in range(E):
        n = len(toks[e])
        ye = res.results[e]["y"].sum(axis=0)            # [D, c]
        contrib = (ye[:, :n] + b2[e][:, None]) * cws[e][None, :]
        outT[:, toks[e]] += contrib
    out = np.ascontiguousarray(outT.T).reshape(B, T, D)

    avg_probs = probs.mean(axis=0, dtype=np.float64)
    aux_loss = np.float32(np.mean((avg_probs - 1.0 / E) ** 2) * E)
    return out, aux_loss


# revision 11
# speedup vs baseline: 1.1703x; 1.1703x over previous
"""Trainium2 Bass kernel for a top-2-of-8 MoE layer (B=4, T=2048, D=1024, F=4096).

Strategy: expert-parallel across the 8 NeuronCores. The router (tiny
8192x1024 @ 1024x8 gemm + softmax + top-2) runs on host; it determines the
token->expert dispatch, i.e. the sharding. Each core is assigned one expert
and receives that expert's routed tokens gathered and pre-transposed to
[D, C] layout, plus the expert's w1/b1/w2. On-device, each core runs the
expert FFN for its tokens: h = gelu(x @ w1 + b1); y = h @ w2. The F=4096
dimension is split into 4 resident-weight passes so each pass's weight slabs
(w1[:, fq], w2[fq, :] = 8.4 MB) stay in SBUF while all tokens stream
through; each pass emits a partial y. Host sums the 4 partials, adds b2,
scales by the top-2 combine weight and scatter-adds into the full output.

Matmuls run as float32r (fp32 operands truncated to ~fp22 inside the PE at
full bf16 rate); accumulation is fp32 in PSUM.
"""

import math

import numpy as np

import concourse.bacc as bacc
import concourse.mybir as mybir
import concourse.tile as tile
from concourse import bass_utils

B, T, D, F, E, TOP_K = 4, 2048, 1024, 4096, 8, 2
NT = B * T                      # 8192 tokens
P = 128                         # partitions
NDT = D // P                    # 8 d-tiles
NPASS = 4                       # F split into 4 weight-resident passes
FQ = F // NPASS                 # 1024 features per pass
NFT = FQ // P                   # 8 f-tiles per pass
TB = 512                        # token block (<= one fp32 PSUM bank)

MM_DT = mybir.dt.float32r      # matmul operand dtype (float32r or float32)
F32 = mybir.dt.float32

# Test hooks: when TRACE is set, the SPMD run captures an NTFF profile and the
# BassKernelResults lands in LAST for exec-time inspection.
TRACE = False
TRACE_CORES = [0]
LAST = None


def _blocks(c):
    # Ragged (sub-TB) block first: the pass then ENDS with a full block whose
    # GEMM2 (~27us) covers the next pass's w1 slab reload (~12us).
    out, t0 = [], 0
    r = c % TB
    if r:
        out.append((0, r))
        t0 = r
    while t0 < c:
        out.append((t0, TB))
        t0 += TB
    return out


def build_program(c):
    """One SPMD program; each core runs it on its own expert's data."""
    nc = bacc.Bacc("TRN2", target_bir_lowering=False, debug=False, num_devices=E)

    xt = nc.dram_tensor("xt", [D, c], MM_DT, kind="ExternalInput").ap()
    w1 = nc.dram_tensor("w1", [D, F], MM_DT, kind="ExternalInput").ap()
    b1 = nc.dram_tensor("b1", [P, F // P], F32, kind="ExternalInput").ap()
    w2 = nc.dram_tensor("w2", [F, D], MM_DT, kind="ExternalInput").ap()
    y = nc.dram_tensor("y", [NPASS, D, c], F32, kind="ExternalOutput").ap()

    xt_r = xt.rearrange("(a p) c -> p a c", p=P)    # [128, 8, c]

    with tile.TileContext(nc) as tc:
        with (
            tc.tile_pool(name="w1pool", bufs=2) as w1pool,
            tc.tile_pool(name="w2pool", bufs=1) as w2pool,
            tc.tile_pool(name="bpool", bufs=2) as bpool,
            tc.tile_pool(name="x0pool", bufs=1) as x0pool,
            tc.tile_pool(name="xpool", bufs=2) as xpool,
            tc.tile_pool(name="hpool", bufs=2) as hpool,
            tc.tile_pool(name="ypool", bufs=3) as ypool,
            tc.tile_pool(name="psh", bufs=2, space="PSUM") as pshp,
            tc.tile_pool(name="psy", bufs=4, space="PSUM") as psyp,
        ):
            # The first (ragged) token block is identical in every pass: keep
            # it resident so pass boundaries need no xts DMA at all.
            blks = _blocks(c)
            t00, tb0 = blks[0]
            xts0 = x0pool.tile([P, NDT, tb0], MM_DT)
            nc.scalar.dma_start(xts0[:], xt_r[:, :, t00:t00 + tb0])

            for p in range(NPASS):
                # Resident weight slabs for this pass's feature quarter.
                # w1 on the sync HWDGE ring; w2 follows it (needed ~27us
                # later); xts rides the scalar HWDGE ring in parallel.
                w1s = w1pool.tile([P, NDT, FQ], MM_DT)      # [128, di, f]
                for di in range(NDT):
                    nc.sync.dma_start(
                        w1s[:, di, :],
                        w1[di * P:(di + 1) * P, p * FQ:(p + 1) * FQ],
                    )
                b1s = bpool.tile([P, NFT], F32)
                nc.gpsimd.dma_start(b1s[:], b1[:, p * NFT:(p + 1) * NFT])
                w2s = w2pool.tile([P, NFT, D], MM_DT)       # [128, fi, d]
                for fi in range(NFT):
                    nc.sync.dma_start(
                        w2s[:, fi, :],
                        w2[p * FQ + fi * P:p * FQ + (fi + 1) * P, :],
                    )

                for bi, (t0, tb) in enumerate(blks):
                    if bi == 0:
                        xts = xts0
                    else:
                        xts = xpool.tile([P, NDT, tb], MM_DT, tag="xts")
                        nc.scalar.dma_start(xts[:], xt_r[:, :, t0:t0 + tb])

                    h_all = hpool.tile([P, NFT, tb], MM_DT, tag="h")
                    for fi in range(NFT):
                        psh = pshp.tile([P, tb], F32, tag="psh")
                        for di in range(NDT):
                            nc.tensor.matmul(
                                psh[:],
                                w1s[:, di, fi * P:(fi + 1) * P],
                                xts[:, di, :],
                                start=(di == 0),
                                stop=(di == NDT - 1),
                            )
                        nc.scalar.activation(
                            h_all[:, fi, :],
                            psh[:],
                            mybir.ActivationFunctionType.Gelu,
                            bias=b1s[:, fi:fi + 1],
                        )

                    for dj in range(NDT):
                        psy = psyp.tile([P, tb], F32, tag="psy")
                        for fi in range(NFT):
                            nc.tensor.matmul(
                                psy[:],
                                w2s[:, fi, dj * P:(dj + 1) * P],
                                h_all[:, fi, :],
                                start=(fi == 0),
                                stop=(fi == NFT - 1),
                            )
                        yt = ypool.tile([P, tb], F32, tag="yt")
                        nc.vector.tensor_copy(yt[:], psy[:])
                        nc.gpsimd.dma_start(
                            y[p, dj * P:(dj + 1) * P, t0:t0 + tb], yt[:]
                        )

    nc.compile()
    return nc


def _route(xf, gate_w):
    """Host router: softmax top-2, reference-equivalent in fp32."""
    logits = xf @ gate_w                                   # [NT, E] f32
    m = logits.max(axis=-1, keepdims=True)
    p = np.exp(logits - m, dtype=np.float32)
    p /= p.sum(axis=-1, keepdims=True)
    idx = np.argsort(-p, axis=-1, kind="stable")[:, :TOP_K]  # matches lax.top_k
    wtop = np.take_along_axis(p, idx, axis=-1)
    wtop = wtop / (wtop.sum(axis=-1, keepdims=True) + 1e-9)
    return p, idx, wtop


def kernel(x, gate_w, w1, b1, w2, b2):
    x = np.ascontiguousarray(x, dtype=np.float32)
    gate_w = np.ascontiguousarray(gate_w, dtype=np.float32)
    w1 = np.ascontiguousarray(w1, dtype=np.float32)
    b1 = np.ascontiguousarray(b1, dtype=np.float32)
    w2 = np.ascontiguousarray(w2, dtype=np.float32)
    b2 = np.ascontiguousarray(b2, dtype=np.float32)

    xf = x.reshape(NT, D)
    probs, idx, wtop = _route(xf, gate_w)

    toks, cws = [], []
    for e in range(E):
        sel = (idx[:, 0] == e) | (idx[:, 1] == e)
        tok = np.nonzero(sel)[0]
        cw = np.where(idx[:, 0] == e, wtop[:, 0], wtop[:, 1])[tok]
        toks.append(tok)
        cws.append(cw.astype(np.float32))

    max_n = max(len(t) for t in toks)
    c = max(512, int(math.ceil(max_n / 256.0)) * 256)

    xfT = np.ascontiguousarray(xf.T)                    # [D, NT]
    in_maps = []
    for e in range(E):
        n = len(toks[e])
        xt_e = np.zeros((D, c), dtype=np.float32)
        xt_e[:, :n] = xfT[:, toks[e]]
        in_maps.append({
            "xt": xt_e,
            "w1": w1[e],
            "b1": np.ascontiguousarray(b1[e].reshape(F // P, P).T),
            "w2": w2[e],
        })

    nc = build_program(c)
    global LAST
    res = bass_utils.run_bass_kernel_spmd(
        nc,
        in_maps,
        core_ids=list(range(E)),
        trace=TRACE,
        trace_cores=TRACE_CORES if TRACE else None,
    )
    LAST = res

    outT = np.zeros((D, NT), dtype=np.float32)
    for e in range(E):
        n = len(toks[e])
        ye = res.results[e]["y"].sum(axis=0)            # [D, c]
        contrib = (ye[:, :n] + b2[e][:, None]) * cws[e][None, :]
        outT[:, toks[e]] += contrib
    out = np.ascontiguousarray(outT.T).reshape(B, T, D)

    avg_probs = probs.mean(axis=0, dtype=np.float64)
    aux_loss = np.float32(np.mean((avg_probs - 1.0 / E) ** 2) * E)
    return out, aux_loss


# revision 12
# speedup vs baseline: 1.2241x; 1.0460x over previous
"""Trainium2 Bass kernel for a top-2-of-8 MoE layer (B=4, T=2048, D=1024, F=4096).

Strategy: expert-parallel across the 8 NeuronCores. The router (tiny
8192x1024 @ 1024x8 gemm + softmax + top-2) runs on host; it determines the
token->expert dispatch, i.e. the sharding. Each core is assigned one expert
and receives that expert's routed tokens gathered and pre-transposed to
[D, C] layout, plus the expert's w1/b1/w2. On-device, each core runs the
expert FFN for its tokens: h = gelu(x @ w1 + b1); y = h @ w2. The F=4096
dimension is split into 4 resident-weight passes so each pass's weight slabs
(w1[:, fq], w2[fq, :] = 8.4 MB) stay in SBUF while all tokens stream
through; each pass emits a partial y. Host sums the 4 partials, adds b2,
scales by the top-2 combine weight and scatter-adds into the full output.

Matmuls run as float32r (fp32 operands truncated to ~fp22 inside the PE at
full bf16 rate); accumulation is fp32 in PSUM.
"""

import math

import numpy as np

import concourse.bacc as bacc
import concourse.mybir as mybir
import concourse.tile as tile
from concourse import bass_utils

B, T, D, F, E, TOP_K = 4, 2048, 1024, 4096, 8, 2
NT = B * T                      # 8192 tokens
P = 128                         # partitions
NDT = D // P                    # 8 d-tiles
NPASS = 4                       # F split into 4 weight-resident passes
FQ = F // NPASS                 # 1024 features per pass
NFT = FQ // P                   # 8 f-tiles per pass
TB = 512                        # token block (<= one fp32 PSUM bank)

MM_DT = mybir.dt.float32r      # matmul operand dtype (float32r or float32)
F32 = mybir.dt.float32

# Test hooks: when TRACE is set, the SPMD run captures an NTFF profile and the
# BassKernelResults lands in LAST for exec-time inspection.
TRACE = False
TRACE_CORES = [0]
LAST = None


def _blocks(c):
    # Ragged (sub-TB) block first: the pass then ENDS with a full block whose
    # GEMM2 (~27us) covers the next pass's w1 slab reload (~12us).
    out, t0 = [], 0
    r = c % TB
    if r:
        out.append((0, r))
        t0 = r
    while t0 < c:
        out.append((t0, TB))
        t0 += TB
    return out


def build_program(c):
    """One SPMD program; each core runs it on its own expert's data."""
    nc = bacc.Bacc("TRN2", target_bir_lowering=False, debug=False, num_devices=E)

    xt = nc.dram_tensor("xt", [D, c], MM_DT, kind="ExternalInput").ap()
    w1 = nc.dram_tensor("w1", [D, F], MM_DT, kind="ExternalInput").ap()
    b1 = nc.dram_tensor("b1", [P, F // P], F32, kind="ExternalInput").ap()
    w2 = nc.dram_tensor("w2", [F, D], MM_DT, kind="ExternalInput").ap()
    y = nc.dram_tensor("y", [NPASS, D, c], F32, kind="ExternalOutput").ap()

    xt_r = xt.rearrange("(a p) c -> p a c", p=P)    # [128, 8, c]

    with tile.TileContext(nc) as tc:
        with (
            tc.tile_pool(name="w1pool", bufs=1) as w1pool,
            tc.tile_pool(name="w2pool", bufs=1) as w2pool,
            tc.tile_pool(name="bpool", bufs=2) as bpool,
            tc.tile_pool(name="x0pool", bufs=1) as x0pool,
            tc.tile_pool(name="xpool", bufs=1) as xpool,
            tc.tile_pool(name="hpool", bufs=2) as hpool,
            tc.tile_pool(name="ypool", bufs=3) as ypool,
            tc.tile_pool(name="psh", bufs=2, space="PSUM") as pshp,
            tc.tile_pool(name="psy", bufs=4, space="PSUM") as psyp,
        ):
            # The first (ragged) token block is identical in every pass: keep
            # it resident so pass boundaries need no xts DMA at all.
            blks = _blocks(c)
            t00, tb0 = blks[0]
            xts0 = x0pool.tile([P, NDT, tb0], MM_DT)
            nc.scalar.dma_start(xts0[:], xt_r[:, :, t00:t00 + tb0])

            for p in range(NPASS):
                # Resident weight slabs for this pass's feature quarter.
                # w1 on the sync HWDGE ring; w2 follows it (needed ~27us
                # later); xts rides the scalar HWDGE ring in parallel.
                # Alternating tags pin even/odd passes to distinct SBUF
                # regions, so the next pass's slabs prefetch with no
                # write-after-read wait on the previous pass's matmuls.
                w1s = w1pool.tile([P, NDT, FQ], MM_DT, tag=f"w1{p % 2}")
                for di in range(NDT):
                    nc.sync.dma_start(
                        w1s[:, di, :],
                        w1[di * P:(di + 1) * P, p * FQ:(p + 1) * FQ],
                    )
                b1s = bpool.tile([P, NFT], F32)
                nc.gpsimd.dma_start(b1s[:], b1[:, p * NFT:(p + 1) * NFT])
                w2s = w2pool.tile([P, NFT, D], MM_DT, tag=f"w2{p % 2}")
                for fi in range(NFT):
                    nc.sync.dma_start(
                        w2s[:, fi, :],
                        w2[p * FQ + fi * P:p * FQ + (fi + 1) * P, :],
                    )

                for bi, (t0, tb) in enumerate(blks):
                    if bi == 0:
                        xts = xts0
                    else:
                        xts = xpool.tile([P, NDT, tb], MM_DT, tag="xts")
                        nc.scalar.dma_start(xts[:], xt_r[:, :, t0:t0 + tb])

                    h_all = hpool.tile([P, NFT, tb], MM_DT, tag="h")
                    for fi in range(NFT):
                        psh = pshp.tile([P, tb], F32, tag="psh")
                        for di in range(NDT):
                            nc.tensor.matmul(
                                psh[:],
                                w1s[:, di, fi * P:(fi + 1) * P],
                                xts[:, di, :],
                                start=(di == 0),
                                stop=(di == NDT - 1),
                            )
                        nc.scalar.activation(
                            h_all[:, fi, :],
                            psh[:],
                            mybir.ActivationFunctionType.Gelu,
                            bias=b1s[:, fi:fi + 1],
                        )

                    for dj in range(NDT):
                        psy = psyp.tile([P, tb], F32, tag="psy")
                        for fi in range(NFT):
                            nc.tensor.matmul(
                                psy[:],
                                w2s[:, fi, dj * P:(dj + 1) * P],
                                h_all[:, fi, :],
                                start=(fi == 0),
                                stop=(fi == NFT - 1),
                            )
                        yt = ypool.tile([P, tb], F32, tag="yt")
                        nc.vector.tensor_copy(yt[:], psy[:])
                        nc.gpsimd.dma_start(
                            y[p, dj * P:(dj + 1) * P, t0:t0 + tb], yt[:]
                        )

    nc.compile()
    return nc


def _route(xf, gate_w):
    """Host router: softmax top-2, reference-equivalent in fp32."""
    logits = xf @ gate_w                                   # [NT, E] f32
    m = logits.max(axis=-1, keepdims=True)
    p = np.exp(logits - m, dtype=np.float32)
    p /= p.sum(axis=-1, keepdims=True)
    idx = np.argsort(-p, axis=-1, kind="stable")[:, :TOP_K]  # matches lax.top_k
    wtop = np.take_along_axis(p, idx, axis=-1)
    wtop = wtop / (wtop.sum(axis=-1, keepdims=True) + 1e-9)
    return p, idx, wtop


def kernel(x, gate_w, w1, b1, w2, b2):
    x = np.ascontiguousarray(x, dtype=np.float32)
    gate_w = np.ascontiguousarray(gate_w, dtype=np.float32)
    w1 = np.ascontiguousarray(w1, dtype=np.float32)
    b1 = np.ascontiguousarray(b1, dtype=np.float32)
    w2 = np.ascontiguousarray(w2, dtype=np.float32)
    b2 = np.ascontiguousarray(b2, dtype=np.float32)

    xf = x.reshape(NT, D)
    probs, idx, wtop = _route(xf, gate_w)

    toks, cws = [], []
    for e in range(E):
        sel = (idx[:, 0] == e) | (idx[:, 1] == e)
        tok = np.nonzero(sel)[0]
        cw = np.where(idx[:, 0] == e, wtop[:, 0], wtop[:, 1])[tok]
        toks.append(tok)
        cws.append(cw.astype(np.float32))

    max_n = max(len(t) for t in toks)
    c = max(512, int(math.ceil(max_n / 256.0)) * 256)

    xfT = np.ascontiguousarray(xf.T)                    # [D, NT]
    in_maps = []
    for e in range(E):
        n = len(toks[e])
        xt_e = np.zeros((D, c), dtype=np.float32)
        xt_e[:, :n] = xfT[:, toks[e]]
        in_maps.append({
            "xt": xt_e,
            "w1": w1[e],
            "b1": np.ascontiguousarray(b1[e].reshape(F // P, P).T),
            "w2": w2[e],
        })

    nc = build_program(c)
    global LAST
    res = bass_utils.run_bass_kernel_spmd(
        nc,
        in_maps,
        core_ids=list(range(E)),
        trace=TRACE,
        trace_cores=TRACE_CORES if TRACE else None,
    )
    LAST = res

    outT = np.zeros((D, NT), dtype=np.float32)
    for e in range(E):
        n = len(toks[e])
        ye = res.results[e]["y"].sum(axis=0)            # [D, c]
        contrib = (ye[:, :n] + b2[e][:, None]) * cws[e][None, :]
        outT[:, toks[e]] += contrib
    out = np.ascontiguousarray(outT.T).reshape(B, T, D)

    avg_probs = probs.mean(axis=0, dtype=np.float64)
    aux_loss = np.float32(np.mean((avg_probs - 1.0 / E) ** 2) * E)
    return out, aux_loss
